# revision 11
# baseline (speedup 1.0000x reference)
"""Trainium2 Bass kernel for AdditiveAttention scores.

Math:  scores[q, k] = sum_d v[d] * tanh(q_t[q, d] + k_t[k, d])
       q_t = query @ W_q.T   [512, 256]
       k_t = key   @ W_k.T   [1024, 256]

Sharding: query (Tq=512) split across 8 cores (64 rows each); key / weights / v
replicated.  No collectives needed — each core produces its own 64 rows of the
[512, 1024] score matrix.

Per-core dataflow (all on-chip after the initial DMAs):
  1. PE (fp32): k_tT[dout, k] and q_tT[dout, q] projections via accumulating
     matmuls (contraction over d_in on partitions).
  2. DVE: biased[d, j*Tk + k] = k_tT[d, k] + q_tT[d, q]  — tensor_scalar_add
     with a per-partition scalar (2x fp32 mode), batched 8 q's wide so the
     ACT instruction overhead amortizes.
  3. ACT: tanh over [128, 8192] tiles (the bottleneck engine: 1 elem/lane/cyc),
     output in bf16.
  4. PE (bf16): per-q dot with v via M=1 matmuls; 4 q's land on PSUM
     partitions {0, 32, 64, 96} (col tile_position), accumulated over the two
     d-halves.
  5. DMA: strided PSUM read (partition step 32) straight to the DRAM output.
"""

import sys

import numpy as np

for _p in ("/opt/trn_rl_repo",):
    if _p not in sys.path:
        sys.path.append(_p)

import concourse.bass as bass
import concourse.tile as tile
from concourse import bacc
from concourse import mybir
from concourse.bass_utils import run_bass_kernel_spmd

N_CORES = 8
TQ, TK, D = 512, 1024, 256
TQ_SH = TQ // N_CORES  # 64 query rows per core
P = 128                # SBUF partitions
ND = D // P            # 2 d-halves
GQ = 16                # q rows batched per ACT instruction group
GROUPS = TQ_SH // GQ   # 8 groups
F32 = mybir.dt.float32
BF16 = mybir.dt.bfloat16
TANH = mybir.ActivationFunctionType.Tanh


def build_bass() -> bass.Bass:
    from contextlib import ExitStack

    nc = bacc.Bacc()
    qT = nc.declare_dram_parameter("qT", [D, TQ_SH], F32, isOutput=False)
    kT = nc.declare_dram_parameter("kT", [D, TK], F32, isOutput=False)
    wqT = nc.declare_dram_parameter("wqT", [D, D], F32, isOutput=False)
    wkT = nc.declare_dram_parameter("wkT", [D, D], F32, isOutput=False)
    v = nc.declare_dram_parameter("v", [D, 1], F32, isOutput=False)
    out = nc.declare_dram_parameter("out", [TQ_SH, TK], F32, isOutput=True)

    with tile.TileContext(nc) as tc, ExitStack() as ctx:
        # persistent tiles: k_tT (bf16), q_tT (f32), v (bf16)
        consts = ctx.enter_context(tc.tile_pool(name="consts", bufs=1))
        ktt_sb = [consts.tile([P, TK], BF16, tag=f"ktt{m}", name=f"ktt{m}") for m in range(ND)]
        qtt_sb = [consts.tile([P, TQ_SH], F32, tag=f"qtt{m}", name=f"qtt{m}") for m in range(ND)]
        v_bf = [consts.tile([P, 1], BF16, tag=f"vbf{i}", name=f"v_bf{i}") for i in range(ND)]

        # projection phase: inputs live only here (pool closes -> SBUF freed)
        with tc.tile_pool(name="pin", bufs=1) as pin, \
             tc.tile_pool(name="ppsum", bufs=2, space="PSUM") as ppool:
            kT_sb, qT_sb, wqT_sb, wkT_sb = [], [], [], []
            for i in range(ND):
                t = pin.tile([P, TK], F32, tag=f"kT{i}", name=f"kT_sb{i}")
                nc.sync.dma_start(t[:], kT[i * P : (i + 1) * P, :])
                kT_sb.append(t)
                t = pin.tile([P, TQ_SH], F32, tag=f"qT{i}", name=f"qT_sb{i}")
                nc.sync.dma_start(t[:], qT[i * P : (i + 1) * P, :])
                qT_sb.append(t)
                t = pin.tile([P, D], F32, tag=f"wqT{i}", name=f"wqT_sb{i}")
                nc.sync.dma_start(t[:], wqT[i * P : (i + 1) * P, :])
                wqT_sb.append(t)
                t = pin.tile([P, D], F32, tag=f"wkT{i}", name=f"wkT_sb{i}")
                nc.sync.dma_start(t[:], wkT[i * P : (i + 1) * P, :])
                wkT_sb.append(t)
                t = pin.tile([P, 1], F32, tag=f"v{i}", name=f"v_sb{i}")
                nc.sync.dma_start(t[:], v[i * P : (i + 1) * P, :])
                nc.vector.tensor_copy(v_bf[i][:], t[:])

            for m in range(ND):
                for c in range(TK // 512):
                    ps = ppool.tile([P, 512], F32, tag="pk", name="pk", bufs=4)
                    for k in range(ND):
                        nc.tensor.matmul(
                            ps[:],
                            lhsT=wkT_sb[k][:, m * P : (m + 1) * P],
                            rhs=kT_sb[k][:, c * 512 : (c + 1) * 512],
                            start=(k == 0),
                            stop=(k == ND - 1),
                        )
                    nc.vector.tensor_copy(ktt_sb[m][:, c * 512 : (c + 1) * 512], ps[:])
                ps = ppool.tile([P, TQ_SH], F32, tag="pq", name="pq", bufs=2)
                for k in range(ND):
                    nc.tensor.matmul(
                        ps[:],
                        lhsT=wqT_sb[k][:, m * P : (m + 1) * P],
                        rhs=qT_sb[k][:, :],
                        start=(k == 0),
                        stop=(k == ND - 1),
                    )
                nc.vector.tensor_copy(qtt_sb[m][:, :], ps[:])

        # ---- main loop: bias-add (DVE 4x bf16) -> tanh (ACT) -> v-dot (PE) ----
        bias_pool = ctx.enter_context(tc.tile_pool(name="biased", bufs=2))
        tanh_pool = ctx.enter_context(tc.tile_pool(name="tanh", bufs=3))
        mpsum = ctx.enter_context(tc.tile_pool(name="mpsum", bufs=4, space="PSUM"))
        stage_pool = ctx.enter_context(tc.tile_pool(name="stage", bufs=3))

        for g in range(GROUPS):
            tanh_t = []
            for dt in range(ND):
                b = bias_pool.tile([P, GQ * TK], BF16, tag="biased", name="biased")
                for j in range(GQ):
                    q = g * GQ + j
                    nc.vector.tensor_scalar_add(
                        b[:, j * TK : (j + 1) * TK],
                        ktt_sb[dt][:],
                        qtt_sb[dt][:, q : q + 1],
                    )
                th = tanh_pool.tile([P, GQ * TK], BF16, tag="tanh", name="tanh_t")
                nc.scalar.activation(th[:], b[:], TANH)
                tanh_t.append(th)
            # sub-groups of 4 q's; each lands on PSUM partitions {0,32,64,96}
            for h in range(GQ // 4):
                ps = mpsum.tile([P, TK], F32, tag="mps", name="mps")
                for j4 in range(4):
                    j = h * 4 + j4
                    for c in range(TK // 512):
                        o = ps[32 * j4 : 32 * j4 + 1, c * 512 : (c + 1) * 512]
                        for dt in range(ND):
                            nc.tensor.matmul(
                                o,
                                lhsT=v_bf[dt][:, 0:1],
                                rhs=tanh_t[dt][
                                    :, j * TK + c * 512 : j * TK + (c + 1) * 512
                                ],
                                start=(dt == 0),
                                stop=(dt == ND - 1),
                                tile_position=(0, 32 * j4),
                            )
                q0 = g * GQ + h * 4
                stage = stage_pool.tile([P, TK], F32, tag="stage", name="stage")
                nc.vector.tensor_copy(stage[:, :], ps[:, :])
                nc.sync.dma_start(out[q0 : q0 + 4, :], stage[0:P:32, :])

    nc.finalize()  # Bacc: runs wait-splitting + register-allocation passes
    return nc


def kernel(**inputs: np.ndarray) -> np.ndarray:
    query = np.ascontiguousarray(np.asarray(inputs["query"], dtype=np.float32))
    key = np.asarray(inputs["key"], dtype=np.float32)
    W_q = np.asarray(inputs["W_q"], dtype=np.float32)
    W_k = np.asarray(inputs["W_k"], dtype=np.float32)
    v_w = np.asarray(inputs["v_w"], dtype=np.float32)

    kT = np.ascontiguousarray(key.T)
    wqT = np.ascontiguousarray(W_q.T)
    wkT = np.ascontiguousarray(W_k.T)
    v = np.ascontiguousarray(v_w.reshape(1, D).T)

    nc = build_bass()
    in_maps = []
    for c in range(N_CORES):
        in_maps.append(
            {
                "qT": np.ascontiguousarray(query[c * TQ_SH : (c + 1) * TQ_SH, :].T),
                "kT": kT,
                "wqT": wqT,
                "wkT": wkT,
                "v": v,
            }
        )
    res = run_bass_kernel_spmd(nc, in_maps, core_ids=list(range(N_CORES)))
    out = np.concatenate(
        [res.results[c]["out"] for c in range(N_CORES)], axis=0
    ).astype(np.float32)
    return out


# revision 14
# speedup vs baseline: 1.1368x; 1.1368x over previous
"""Trainium2 Bass kernel for AdditiveAttention scores.

Math:  scores[q, k] = sum_d v[d] * tanh(q_t[q, d] + k_t[k, d])
       q_t = query @ W_q.T   [512, 256]
       k_t = key   @ W_k.T   [1024, 256]

Sharding: query (Tq=512) split across 8 cores (64 rows each); key / weights / v
replicated.  No collectives needed — each core produces its own 64 rows of the
[512, 1024] score matrix.

Per-core dataflow (all on-chip after the initial DMAs):
  1. PE (fp32): k_tT[dout, k] and q_tT[dout, q] projections via accumulating
     matmuls (contraction over d_in on partitions).
  2. DVE: biased[d, j*Tk + k] = k_tT[d, k] + q_tT[d, q]  — tensor_scalar_add
     with a per-partition scalar (2x fp32 mode), batched 8 q's wide so the
     ACT instruction overhead amortizes.
  3. ACT: tanh over [128, 8192] tiles (the bottleneck engine: 1 elem/lane/cyc),
     output in bf16.
  4. PE (bf16): per-q dot with v via M=1 matmuls; 4 q's land on PSUM
     partitions {0, 32, 64, 96} (col tile_position), accumulated over the two
     d-halves.
  5. DMA: strided PSUM read (partition step 32) straight to the DRAM output.
"""

import sys

import numpy as np

for _p in ("/opt/trn_rl_repo",):
    if _p not in sys.path:
        sys.path.append(_p)

import concourse.bass as bass
import concourse.tile as tile
from concourse import bacc
from concourse import mybir
from concourse.bass_utils import run_bass_kernel_spmd

N_CORES = 8
TQ, TK, D = 512, 1024, 256
TQ_SH = TQ // N_CORES  # 64 query rows per core
P = 128                # SBUF partitions
ND = D // P            # 2 d-halves
GROUP_SIZES = [4, 4, 8, 16, 16, 8, 4, 4]  # q rows per ACT group (sum = TQ_SH)
F32 = mybir.dt.float32
BF16 = mybir.dt.bfloat16
TANH = mybir.ActivationFunctionType.Tanh


def build_bass() -> bass.Bass:
    from contextlib import ExitStack

    assert sum(GROUP_SIZES) == TQ_SH
    nc = bacc.Bacc()
    qT = nc.declare_dram_parameter("qT", [D, TQ_SH], BF16, isOutput=False)
    kT = nc.declare_dram_parameter("kT", [D, TK], BF16, isOutput=False)
    wqT = nc.declare_dram_parameter("wqT", [D, D], BF16, isOutput=False)
    wkT = nc.declare_dram_parameter("wkT", [D, D], BF16, isOutput=False)
    v = nc.declare_dram_parameter("v", [D, 1], BF16, isOutput=False)
    out = nc.declare_dram_parameter("out", [TQ_SH, TK], F32, isOutput=True)

    with tile.TileContext(nc) as tc, ExitStack() as ctx:
        # persistent tiles: k_tT (bf16), q_tT (f32), v (bf16)
        consts = ctx.enter_context(tc.tile_pool(name="consts", bufs=1))
        ktt_sb = [consts.tile([P, TK], BF16, tag=f"ktt{m}", name=f"ktt{m}") for m in range(ND)]
        qtt_sb = [consts.tile([P, TQ_SH], F32, tag=f"qtt{m}", name=f"qtt{m}") for m in range(ND)]
        v_bf = [consts.tile([P, 1], BF16, tag=f"vbf{i}", name=f"v_bf{i}") for i in range(ND)]

        # projection phase: inputs live only here (pool closes -> SBUF freed)
        with tc.tile_pool(name="pin", bufs=1) as pin, \
             tc.tile_pool(name="ppsum", bufs=2, space="PSUM") as ppool:
            kT_sb, qT_sb, wqT_sb, wkT_sb = [], [], [], []
            dma_engines = [nc.sync, nc.gpsimd, nc.scalar]
            di = 0

            def dma(dst, src):
                nonlocal di
                dma_engines[di % len(dma_engines)].dma_start(dst, src)
                di += 1

            for i in range(ND):
                t = pin.tile([P, TK], BF16, tag=f"kT{i}", name=f"kT_sb{i}")
                dma(t[:], kT[i * P : (i + 1) * P, :])
                kT_sb.append(t)
                t = pin.tile([P, TQ_SH], BF16, tag=f"qT{i}", name=f"qT_sb{i}")
                dma(t[:], qT[i * P : (i + 1) * P, :])
                qT_sb.append(t)
                t = pin.tile([P, D], BF16, tag=f"wqT{i}", name=f"wqT_sb{i}")
                dma(t[:], wqT[i * P : (i + 1) * P, :])
                wqT_sb.append(t)
                t = pin.tile([P, D], BF16, tag=f"wkT{i}", name=f"wkT_sb{i}")
                dma(t[:], wkT[i * P : (i + 1) * P, :])
                wkT_sb.append(t)
                dma(v_bf[i][:], v[i * P : (i + 1) * P, :])

            # q_tT first (cheap, unblocks the bias adds), then k_tT halves
            for m in range(ND):
                ps = ppool.tile([P, TQ_SH], F32, tag="pq", name="pq", bufs=2)
                for k in range(ND):
                    nc.tensor.matmul(
                        ps[:],
                        lhsT=wqT_sb[k][:, m * P : (m + 1) * P],
                        rhs=qT_sb[k][:, :],
                        start=(k == 0),
                        stop=(k == ND - 1),
                    )
                nc.vector.tensor_copy(qtt_sb[m][:, :], ps[:])
            for m in range(ND):
                for c in range(TK // 512):
                    ps = ppool.tile([P, 512], F32, tag="pk", name="pk", bufs=4)
                    for k in range(ND):
                        nc.tensor.matmul(
                            ps[:],
                            lhsT=wkT_sb[k][:, m * P : (m + 1) * P],
                            rhs=kT_sb[k][:, c * 512 : (c + 1) * 512],
                            start=(k == 0),
                            stop=(k == ND - 1),
                        )
                    nc.vector.tensor_copy(ktt_sb[m][:, c * 512 : (c + 1) * 512], ps[:])

        # ---- main loop: bias-add (DVE 4x bf16) -> tanh (ACT) -> v-dot (PE) ----
        bias_pool = ctx.enter_context(tc.tile_pool(name="biased", bufs=2))
        tanh_pool = ctx.enter_context(tc.tile_pool(name="tanh", bufs=3))
        mpsum = ctx.enter_context(tc.tile_pool(name="mpsum", bufs=4, space="PSUM"))
        stage_pool = ctx.enter_context(tc.tile_pool(name="stage", bufs=3))

        q_base = 0
        for g, gq in enumerate(GROUP_SIZES):
            tanh_t = []
            for dt in range(ND):
                b = bias_pool.tile([P, gq * TK], BF16, tag="biased", name="biased")
                for j in range(gq):
                    q = q_base + j
                    nc.vector.tensor_scalar_add(
                        b[:, j * TK : (j + 1) * TK],
                        ktt_sb[dt][:],
                        qtt_sb[dt][:, q : q + 1],
                    )
                th = tanh_pool.tile([P, gq * TK], BF16, tag="tanh", name="tanh_t")
                nc.scalar.activation(th[:], b[:], TANH)
                tanh_t.append(th)
            # sub-groups of 4 q; dt-major so dt0 matmuls run during the dt1 tanh
            for h in range(gq // 4):
                ps = mpsum.tile([P, TK], F32, tag="mps", name="mps")
                for dt in range(ND):
                    for j4 in range(4):
                        j = h * 4 + j4
                        for c in range(TK // 512):
                            o = ps[32 * j4 : 32 * j4 + 1, c * 512 : (c + 1) * 512]
                            nc.tensor.matmul(
                                o,
                                lhsT=v_bf[dt][:, 0:1],
                                rhs=tanh_t[dt][
                                    :, j * TK + c * 512 : j * TK + (c + 1) * 512
                                ],
                                start=(dt == 0),
                                stop=(dt == ND - 1),
                                tile_position=(0, 32 * j4),
                            )
                q0 = q_base + h * 4
                stage = stage_pool.tile([P, TK], F32, tag="stage", name="stage")
                nc.vector.tensor_copy(stage[:, :], ps[:, :])
                nc.sync.dma_start(out[q0 : q0 + 4, :], stage[0:P:32, :])
            q_base += gq

    nc.finalize()  # Bacc: runs wait-splitting + register-allocation passes
    return nc


def make_in_maps(inputs):
    import ml_dtypes

    BF = ml_dtypes.bfloat16
    query = np.ascontiguousarray(np.asarray(inputs["query"], dtype=np.float32))
    key = np.asarray(inputs["key"], dtype=np.float32)
    W_q = np.asarray(inputs["W_q"], dtype=np.float32)
    W_k = np.asarray(inputs["W_k"], dtype=np.float32)
    v_w = np.asarray(inputs["v_w"], dtype=np.float32)

    kT = np.ascontiguousarray(key.T.astype(BF))
    wqT = np.ascontiguousarray(W_q.T.astype(BF))
    wkT = np.ascontiguousarray(W_k.T.astype(BF))
    v = np.ascontiguousarray(v_w.reshape(1, D).T.astype(BF))

    in_maps = []
    for c in range(N_CORES):
        in_maps.append(
            {
                "qT": np.ascontiguousarray(
                    query[c * TQ_SH : (c + 1) * TQ_SH, :].T.astype(BF)
                ),
                "kT": kT,
                "wqT": wqT,
                "wkT": wkT,
                "v": v,
            }
        )
    return in_maps


def kernel(**inputs: np.ndarray) -> np.ndarray:
    nc = build_bass()
    in_maps = make_in_maps(inputs)
    res = run_bass_kernel_spmd(nc, in_maps, core_ids=list(range(N_CORES)))
    out = np.concatenate(
        [res.results[c]["out"] for c in range(N_CORES)], axis=0
    ).astype(np.float32)
    return out
